# revision 9
# baseline (speedup 1.0000x reference)
"""Trainium2 Bass kernel for nn_Disc_edge_15573551415682 (GNN message passing).

Sharding: data-parallel over batch B=8 -> 8 NeuronCores (1 graph/core).

Per graph, edge tensors live in "pair-tile" layout:
  pair q in [0,128) covers node rows (q, q+128); tile rows p = f + 64r hold
  feature f of row q+128r; tile cols are the neighbor index j.

Per layer, per pair q, ONE fp8 DoubleRow matmul computes the whole edge
update into PSUM [128, 256]:
  MM_ex  (K=128, 2 k-tiles): ktile0 = e-pair window of the e-arena
         (weights blockdiag(We_e; We_e)), ktile1 = shared xblock
         (x^T on rows 0:64, weights [Wxj|Wxj]; rows 64:128 zero).
         The two k-tiles address the SAME arena tile via a per-pair
         stride so ktile1 always lands on the shared xblock columns.
  MM_aux (K=2, 2 k-tiles): rhs strip rows = (A-1 mask row, ones row);
         weights = (BIG=240 mask pattern, per-pair bias Axi+be).
         Masked cols get -240 before relu -> exact 0; the ones row adds
         the sender-node bias, so evictions are bias-free.

Evictions are [128, 2048] relu-only chunks (8 pairs) split across
ACT/DVE, writing fp8 e-arenas. Layer-0 row-sums (agg) come from a bf16
fold-tree over the fp8 e1-arena on DVE (2x all-SBUF rate). Layer-2
eviction accum_out yields the masked column sums; the tiny head MLP
runs on host.
"""

import sys
from contextlib import ExitStack

import numpy as np

sys.path.insert(0, "/opt/trn_rl_repo")

import ml_dtypes  # noqa: E402

import concourse.bacc as bacc  # noqa: E402
import concourse.bass as bass  # noqa: E402
import concourse.tile as tile  # noqa: E402
from concourse import mybir  # noqa: E402
from concourse.bass_utils import run_bass_kernel_spmd  # noqa: E402

BF16 = ml_dtypes.bfloat16
F8 = ml_dtypes.float8_e4m3
F32 = np.float32

B, N, FN, FE = 8, 256, 64, 64
NPAIR = 128
CH = 8                 # pairs per chunk
NCHUNK = NPAIR // CH   # 16
AW = NPAIR * 256       # 32768 e-arena cols
ANC = AW + 256         # + shared xblock slot
BIG = 240.0

_DT = mybir.dt
AP = bass.AP
_nc_cache = None

# eviction engine maps ('A' = ACT, 'V' = DVE), tuned vs sim
L0E = ["A"] * NCHUNK
L1E = ["A", "V", "A", "V", "A", "V", "A", "V",
       "A", "V", "A", "V", "A", "V", "A", "A"]
L2E = ["A", "V", "A", "V", "A", "V", "A", "V",
       "A", "V", "A", "V", "A", "V", "A", "A"]


def _relu(a):
    return np.maximum(a, 0.0)


def _build_program():
    nc = bacc.Bacc(
        "TRN2", target_bir_lowering=False, debug=False, num_devices=8
    )

    def din(name, shape, dt):
        return nc.dram_tensor(name, shape, dt, kind="ExternalInput").ap()

    e0d = din("e0", [128, ANC], _DT.float8e4)
    wexd = [din(f"wex{l}", [128, 256], _DT.float8e4) for l in range(3)]
    auxrd = din("auxr", [6, 22016], _DT.float8e4)
    auxwd = [din(f"auxw{l}", [6, 11008], _DT.float8e4) for l in range(3)]
    cbd = din("cb", [128, 896], _DT.bfloat16)
    vaccd = nc.dram_tensor(
        "vacc", [128, NCHUNK], _DT.float32, kind="ExternalOutput"
    ).ap()

    AF = mybir.ActivationFunctionType
    ALU = mybir.AluOpType
    PM = mybir.MatmulPerfMode.DoubleRow

    with tile.TileContext(nc) as tc, ExitStack() as ctx:
        cst = ctx.enter_context(tc.tile_pool(name="cst", bufs=1))
        arp = ctx.enter_context(tc.tile_pool(name="ar", bufs=1))
        psp = ctx.enter_context(tc.tile_pool(name="psB", bufs=2, space="PSUM"))
        fsc = ctx.enter_context(tc.tile_pool(name="fsc", bufs=2))
        deadp = ctx.enter_context(tc.tile_pool(name="dead", bufs=2))
        smallp = ctx.enter_context(tc.tile_pool(name="small", bufs=1))

        # ---------------- input tiles ----------------
        e0a = arp.tile([128, ANC], _DT.float8e4, tag="e0a", name="e0a")
        e1a = arp.tile([128, ANC], _DT.float8e4, tag="e1a", name="e1a")
        e2a = arp.tile([128, ANC], _DT.float8e4, tag="e2a", name="e2a")

        # DMA order: layer-0-critical consts first (HWDGE), then e0
        # chunks 2..7 via SWDGE while chunks 0..1 ride HWDGE.
        wex = [
            cst.tile([128, 2, 128], _DT.float8e4, tag=f"wex{l}",
                     name=f"wex{l}")
            for l in range(3)
        ]
        auxr = cst.tile([128, 22016], _DT.float8e4, tag="auxr", name="auxr")
        auxw = [
            cst.tile([128, 11008], _DT.float8e4, tag=f"auxw{l}",
                     name=f"auxw_{l}")
            for l in range(3)
        ]
        cb = cst.tile([128, 896], _DT.bfloat16, tag="cb", name="cb")

        nc.sync.dma_start(wex[0][:].rearrange("p a b -> p (a b)"), wexd[0])
        nc.sync.dma_start(e0a[:, AW:ANC], e0d[:, AW:ANC])
        nc.sync.dma_start(auxr[0:2, :], auxrd[0:2, :])
        nc.sync.dma_start(auxw[0][0:2, :], auxwd[0][0:2, :])
        for c in range(2):
            nc.sync.dma_start(
                e0a[:, c * 4096 : (c + 1) * 4096],
                e0d[:, c * 4096 : (c + 1) * 4096],
            )
        for c in range(2, NCHUNK // 2):
            nc.gpsimd.dma_start(
                e0a[:, c * 4096 : (c + 1) * 4096],
                e0d[:, c * 4096 : (c + 1) * 4096],
            )
        for g in range(3):
            if g > 0:
                nc.sync.dma_start(auxr[32 * g : 32 * g + 2, :],
                                  auxrd[2 * g : 2 * g + 2, :])
                nc.sync.dma_start(auxw[0][32 * g : 32 * g + 2, :],
                                  auxwd[0][2 * g : 2 * g + 2, :])
        nc.sync.dma_start(cb[:], cbd)
        for l in (1, 2):
            nc.sync.dma_start(wex[l][:].rearrange("p a b -> p (a b)"), wexd[l])
            for g in range(3):
                nc.sync.dma_start(auxw[l][32 * g : 32 * g + 2, :],
                                  auxwd[l][2 * g : 2 * g + 2, :])
        bias0 = cb[:, 768:896]
        x0t = cb[0:64, 0:256]
        wn0x = cb[0:64, 256:320]
        wn0a = cb[0:64, 320:384]
        wn0a2 = cb[:, 384:448]
        dinvp = cb[:, 448:576]
        wxibe = [cb[0:65, 576:640], cb[0:65, 640:704]]
        bn0c = cb[0:64, 704:705]

        # ---------------- scratch / state ----------------
        aggv = smallp.tile([128, 128], _DT.float32, tag="aggv")
        zeros = smallp.tile([128, 256], _DT.bfloat16, tag="zeros")
        aggs = smallp.tile([128, 128], _DT.bfloat16, tag="aggs")
        x1o = smallp.tile([65, 256], _DT.bfloat16, tag="x1o")
        vacc = smallp.tile([128, NCHUNK], _DT.float32, tag="vacc")
        axt = smallp.tile([128, 256], _DT.float8e4, tag="axt")  # 4x [128,64]

        nc.vector.memset(x1o[64:65, :], 1.0)
        nc.vector.memset(e1a[64:128, AW:ANC], 0.0)
        nc.vector.memset(zeros[:], 0.0)
        # preload ACT function table off the critical path
        nc.scalar.activation(x1o[64:65, 0:1], x1o[64:65, 0:1], AF.Relu)

        def mm_pair(ps_slice, arena, wexl, auxwl, q):
            g = min(q // 43, 2)
            s = q - 43 * g
            rhs = AP(arena[:].tensor, AW,
                     [[ANC, 128], [q * 256 - AW, 2], [1, 256]])
            nc.tensor.matmul(ps_slice, wexl[:], rhs,
                             start=True, stop=False, perf_mode=PM)
            lw = AP(auxwl[:].tensor, 32 * g * 11008 + s * 256,
                    [[11008, 2], [128, 2], [1, 128]])
            lr = AP(auxr[:].tensor, 32 * g * 22016 + s * 512,
                    [[22016, 2], [256, 2], [1, 256]])
            nc.tensor.matmul(ps_slice, lw, lr,
                             start=False, stop=True, perf_mode=PM)

        def evict(eng, dst, ps, acc=None):
            if eng == "A":
                nc.scalar.activation(dst, ps[:], AF.Relu, accum_out=acc)
            else:
                nc.vector.tensor_scalar(dst, ps[:], 0.0, 0.0,
                                        op0=ALU.max, op1=ALU.add,
                                        accum_out=acc)

        # ================= layer 0 =================
        adum = smallp.tile([128, 256], _DT.bfloat16, tag="adum")
        for c in range(NCHUNK):
            ps = psp.tile([128, 2048], _DT.float32, tag="psB", name=f"ps0_{c}")
            for s in range(CH):
                mm_pair(ps[:, s * 256 : (s + 1) * 256], e0a, wex[0],
                        auxw[0], c * CH + s)
            evict(L0E[c], e1a[:, c * 2048 : (c + 1) * 2048], ps)
            # per-pair row-sum pass over the fp8 arena (DVE, 2x all-SBUF;
            # a few pairs go to ACT to balance the engines)
            for s in range(CH):
                q = c * CH + s
                if s == 7 and c % 3 == 2:
                    nc.scalar.activation(
                        adum[:], e1a[:, q * 256 : (q + 1) * 256], AF.Copy,
                        accum_out=aggv[:, q : q + 1])
                else:
                    nc.vector.tensor_scalar(
                        adum[:], e1a[:, q * 256 : (q + 1) * 256], 0.0, 0.0,
                        op0=ALU.add, op1=ALU.add,
                        accum_out=aggv[:, q : q + 1])

        # ================= x1 + aux strips for layers 1/2 =================
        nc.vector.tensor_tensor(aggs[:], aggv[:], dinvp, op=ALU.mult)

        psxa = psp.tile([64, 128], _DT.float32, tag="psB", name="psxa")
        nc.tensor.matmul(psxa[:], wn0x, x0t[:, 0:128], start=True, stop=False)
        nc.tensor.matmul(psxa[:], wn0a, aggs[0:64, :], start=False, stop=True)
        psxb = psp.tile([64, 128], _DT.float32, tag="psB", name="psxb")
        nc.tensor.matmul(psxb[:], wn0x, x0t[:, 128:256], start=True, stop=False)
        nc.tensor.matmul(psxb[:], wn0a2[64:128, :], aggs[64:128, :],
                         start=False, stop=True)
        # x1^T -> e1a xblock slot (fp8, ACT) and x1o (bf16, DVE in parallel)
        nc.scalar.activation(e1a[0:64, AW : AW + 128], psxa[:], AF.Relu,
                             bias=bn0c)
        nc.scalar.activation(e1a[0:64, AW + 128 : AW + 256], psxb[:], AF.Relu,
                             bias=bn0c)
        nc.vector.scalar_tensor_tensor(x1o[0:64, 0:128], psxa[:], bn0c,
                                       zeros[0:64, 0:128],
                                       op0=ALU.add, op1=ALU.max)
        nc.vector.scalar_tensor_tensor(x1o[0:64, 128:256], psxb[:], bn0c,
                                       zeros[0:64, 0:128],
                                       op0=ALU.add, op1=ALU.max)
        nc.vector.tensor_copy(e2a[:, AW:ANC], e1a[:, AW:ANC])

        # per-layer bias strips: Axi_l + be_l, fp8, scattered into auxw[l]
        for li in (1, 2):
            psb = psp.tile([128, 128], _DT.float32, tag="psB", name=f"pb{li}")
            nc.tensor.matmul(psb[:, 0:64], x1o[:, 0:128], wxibe[li - 1],
                             start=True, stop=True)
            nc.tensor.matmul(psb[:, 64:128], x1o[:, 128:256], wxibe[li - 1],
                             start=True, stop=True)
            dst = axt[:, (li - 1) * 128 : li * 128]
            nc.scalar.activation(dst, psb[:], AF.Copy)
            for g, q0, gn in ((0, 0, 43), (1, 43, 43), (2, 86, 42)):
                nc.sync.dma_start(
                    AP(auxw[li][:].tensor, (32 * g + 1) * 11008,
                       [[11008, 1], [256, gn], [1, 128]]),
                    axt[q0 : q0 + gn, (li - 1) * 128 : li * 128],
                )

        # ================= layers 1 and 2 (skewed interleave) =================
        def mm_ex(ps_slice, arena, wexl, q):
            rhs = AP(arena[:].tensor, AW,
                     [[ANC, 128], [q * 256 - AW, 2], [1, 256]])
            nc.tensor.matmul(ps_slice, wexl[:], rhs,
                             start=True, stop=False, perf_mode=PM)

        def mm_aux(ps_slice, auxwl, q):
            g = min(q // 43, 2)
            s = q - 43 * g
            lw = AP(auxwl[:].tensor, 32 * g * 11008 + s * 256,
                    [[11008, 2], [128, 2], [1, 128]])
            lr = AP(auxr[:].tensor, 32 * g * 22016 + s * 512,
                    [[22016, 2], [256, 2], [1, 256]])
            nc.tensor.matmul(ps_slice, lw, lr,
                             start=False, stop=True, perf_mode=PM)

        def l1_chunk(c, split=False):
            ps = psp.tile([128, 2048], _DT.float32, tag="psB", name=f"ps1_{c}")
            if split:
                for s in range(CH):
                    mm_ex(ps[:, s * 256 : (s + 1) * 256], e1a, wex[1],
                          c * CH + s)
                for s in range(CH):
                    mm_aux(ps[:, s * 256 : (s + 1) * 256], auxw[1], c * CH + s)
            else:
                for s in range(CH):
                    mm_pair(ps[:, s * 256 : (s + 1) * 256], e1a, wex[1],
                            auxw[1], c * CH + s)
            evict(L1E[c], e2a[:, c * 2048 : (c + 1) * 2048], ps)

        def l2_chunk(c):
            ps = psp.tile([128, 2048], _DT.float32, tag="psB", name=f"ps2_{c}")
            for s in range(CH):
                mm_pair(ps[:, s * 256 : (s + 1) * 256], e2a, wex[2],
                        auxw[2], c * CH + s)
            dead = deadp.tile([128, 2048], _DT.float8e4, tag="dead",
                              name=f"dead_{c}")
            evict(L2E[c], dead[:], ps, acc=vacc[:, c : c + 1])

        for c in range(NCHUNK + 1):
            if c < NCHUNK:
                l1_chunk(c, split=(c < 2))
            if c >= 1:
                l2_chunk(c - 1)

        nc.sync.dma_start(vaccd, vacc[:])

    nc.compile()
    return nc


def _get_nc():
    global _nc_cache
    if _nc_cache is None:
        _nc_cache = _build_program()
    return _nc_cache


def _prep_core_inputs(b, edge_index, x, edge_attr, weights):
    (We0, be0, Wn0, bn0, We1, be1, We2, be2) = weights
    A = edge_index[b].astype(F32)
    x0 = x[b].astype(F32)

    # e0 arena + xblock0
    e0 = np.empty((128, ANC), F32)
    e0[:, 0:AW] = (
        edge_attr[b].astype(F32)
        .reshape(2, 128, 256, FE)
        .transpose(0, 3, 1, 2)
        .reshape(128, AW)
    )
    e0[:, AW:ANC] = 0.0
    e0[0:64, AW:ANC] = x0.T

    def mk_wex(We):
        # ktile0 = shared xblock (Wxj), ktile1 = e-pair window (We_e)
        out = np.zeros((128, 2, 128), F32)
        Wee = We[128:192]
        out[0:64, 0, 0:64] = We[64:128]
        out[0:64, 0, 64:128] = We[64:128]
        out[0:64, 1, 0:64] = Wee
        out[64:128, 1, 64:128] = Wee
        return out.reshape(128, 256).astype(F8)

    # aux rhs strip: row 2g = (A-1) interleaved, row 2g+1 = ones
    GQ0 = (0, 43, 86)
    GN = (43, 43, 42)
    auxr = np.zeros((6, 22016), F32)
    for g in range(3):
        qs = np.arange(GQ0[g], GQ0[g] + GN[g])
        blk = np.stack([A[qs] - 1.0, A[qs + 128] - 1.0], axis=1)
        auxr[2 * g, 0 : GN[g] * 512] = blk.reshape(-1)
        auxr[2 * g + 1] = 1.0

    def mk_auxw(masked, bias):
        # bias: [256, 64] (Axi + be) or None
        out = np.zeros((6, 43, 2, 128), F32)
        if masked:
            out[0::2, :, 0, 0:64] = BIG
            out[0::2, :, 1, 64:128] = BIG
        if bias is not None:
            for g in range(3):
                qs = np.arange(GQ0[g], GQ0[g] + GN[g])
                out[2 * g + 1, 0 : GN[g], 0, 0:64] = bias[qs]
                out[2 * g + 1, 0 : GN[g], 0, 64:128] = bias[qs + 128]
        return out.reshape(6, 11008).astype(F8)

    bias0 = x0 @ We0[0:64] + be0[None, :]       # [256, 64]
    bias0p = np.concatenate([bias0[0:128].T, bias0[128:256].T], 0)  # [128,128]

    deg = np.clip(A.sum(1), 1.0, None)
    dinv = (1.0 / deg).astype(F32)
    dinvp = np.concatenate(
        [np.tile(dinv[None, 0:128], (64, 1)),
         np.tile(dinv[None, 128:256], (64, 1))], 0
    )

    cb = np.zeros((128, 896), F32)
    cb[0:64, 0:256] = x0.T
    cb[0:64, 256:320] = Wn0[0:64]
    cb[0:64, 320:384] = Wn0[64:128]
    cb[64:128, 384:448] = Wn0[64:128]
    cb[:, 448:576] = dinvp
    cb[0:64, 576:640] = We1[0:64]
    cb[64, 576:640] = be1
    cb[0:64, 640:704] = We2[0:64]
    cb[64, 640:704] = be2
    cb[0:64, 704] = bn0
    cb[:, 768:896] = bias0p

    return {
        "e0": e0.astype(F8),
        "wex0": mk_wex(We0),
        "wex1": mk_wex(We1),
        "wex2": mk_wex(We2),
        "auxr": auxr.astype(F8),
        "auxw0": mk_auxw(True, bias0),
        "auxw1": mk_auxw(False, None),
        "auxw2": mk_auxw(True, None),
        "cb": cb.astype(BF16),
    }


def _finish(res, W1, b1, W2, b2, W3, b3):
    out = np.zeros((B,), F32)
    for b in range(B):
        vacc = res.results[b]["vacc"].astype(F32)
        vcols = vacc.sum(1)
        v = (vcols[0:64] + vcols[64:128]) / float(N * N)
        h = _relu(v @ np.asarray(W1, F32) + np.asarray(b1, F32))
        h = _relu(h @ np.asarray(W2, F32) + np.asarray(b2, F32))
        out[b] = (h @ np.asarray(W3, F32) + np.asarray(b3, F32))[0]
    return out


def _run(edge_index, x, edge_attr, weights, trace=False):
    nc = _get_nc()
    in_maps = [
        _prep_core_inputs(b, np.asarray(edge_index), np.asarray(x),
                          np.asarray(edge_attr), weights)
        for b in range(B)
    ]
    return run_bass_kernel_spmd(nc, in_maps, core_ids=list(range(B)),
                                trace=trace)


def run_traced(edge_index, x, edge_attr,
               We0, be0, Wn0, bn0,
               We1, be1, Wn1, bn1,
               We2, be2, Wn2, bn2,
               W1, b1, W2, b2, W3, b3, **kw):
    weights = tuple(
        np.asarray(w, F32)
        for w in (We0, be0, Wn0, bn0, We1, be1, We2, be2)
    )
    return _run(edge_index, x, edge_attr, weights, trace=True)


def kernel(edge_index, x, edge_attr,
           We0, be0, Wn0, bn0,
           We1, be1, Wn1, bn1,
           We2, be2, Wn2, bn2,
           W1, b1, W2, b2, W3, b3, **kw):
    weights = tuple(
        np.asarray(w, F32)
        for w in (We0, be0, Wn0, bn0, We1, be1, We2, be2)
    )
    res = _run(edge_index, x, edge_attr, weights)
    return _finish(res, W1, b1, W2, b2, W3, b3)
